# revision 34
# baseline (speedup 1.0000x reference)
"""KindredAttention on 8 trn2 NeuronCores.

Sharding: core(b, g) = b*2 + g for batch b in 0..3, head-group g in 0..1
(8 heads per group). Data-parallel over batch, tensor-parallel over heads
(qkv column-split, o_proj row-split; host sums the two o_proj partials).

Per-core layouts (host pre-transposes; all inputs bf16):
  xt   [1024, 2048] bf16 : hidden[b].T                  (d-major)
  wqkv [1024, 1536] bf16 : qkv_w rows for this group, transposed.
                           cols = [q(8 heads x 64) | k(...) | v(...)]
  owt  [512, 1024]  bf16 : o_w[:, group cols].T
  cos/sina [128, 2048] bf16 : RoPE tables (2 heads stacked, sign-folded sin)
  out  [2048, 1024] f32  : partial o_proj output (host adds g=0 + g=1)

Engine budget per core (cost model): PE ~275us is the floor
(proj 82 + QK 109 + PV 55 + o_proj 27us).  The 33.5M-score exp is
split: ACT table-exp on 10/16 k-chunks (as [128,1024] pair ops that
amortize ACT's fixed overhead), DVE Schraudolph int16-exp (bitcast as
bf16) on 6/16.  GPSIMD/Pool cannot touch PSUM on real hw, so ACT
drains qkv-proj PSUM to SBUF (idle in phase 1) and Pool runs the RoPE
rotate/cos muls from there (sina tables pre-shifted +-32 partitions:
both SBUF operands of a TensorTensor must share a base partition);
the combine-add is DVE.  PV is es-stationary (lhsT = scores, moving =
v, 65-wide output) which halves its PE cost vs v-stationary; the
softmax denominator rides a ones-column in v; normalize is
reciprocal + per-partition tensor_scalar into [qpos, d] bf16, and the
o_proj lhsT layout [d(head pair), qpos] comes from a SBUF->SBUF DMA
transpose (XBAR).  o_proj drains ride ACT (idle in o_proj windows).
A one-block software pipeline defers each block's PV behind the next
block's QK/exp; o_proj is emitted per completed q-block.  A single
PSUM pool serves qkv-proj and the ACT exp pairs, so there is no pool
barrier between phases: banks = 2x[128,1024] pairs + 2x[128,512]
Schraudolph singles + 2x[128,512] shared pv/o_proj accumulators.
"""

import os

import ml_dtypes
import numpy as np

H = 16
D = 64
BASE = 10000.0
B, S, HD = 4, 2048, 1024
G = 2          # head groups (tensor parallel)
HG = H // G    # heads per group = 8
N_CORES = 8

# Schraudolph exp in bf16-bit space: exp(x) ~= bitcast_bf16(int16(
#   x * (2^7/ln2) + (127*2^7 - C16))).  C16 calibrated for floor();
# ~1.8% rms, ~4.2% max rel err on the scores this problem produces.
A16 = 128.0 / float(np.log(2.0))
C16 = 7.0
B16 = 127.0 * 128.0 - C16

last_results = None  # stash for test.py (exec_time_ns etc.)


def _rope_tables():
    inv_freq = 1.0 / (BASE ** (np.arange(0, D, 2, dtype=np.float32) / D))
    t = np.arange(S, dtype=np.float32)
    freqs = np.outer(t, inv_freq)                       # [S, 32]
    emb = np.concatenate([freqs, freqs], -1)            # [S, 64]
    cos = np.cos(emb).T.astype(np.float32)              # [64, S]
    sin = np.sin(emb).T.astype(np.float32)
    sina = sin.copy()
    sina[:32] = -sina[:32]                              # rotate_half sign fold
    cos128 = np.tile(cos, (2, 1)).astype(ml_dtypes.bfloat16)    # [128, S]
    sina128 = np.tile(sina, (2, 1)).astype(ml_dtypes.bfloat16)
    sdn = np.roll(sina128, 32, axis=0)    # row r holds sina[r-32]
    sup = np.roll(sina128, -32, axis=0)   # row r holds sina[r+32]
    return (np.ascontiguousarray(cos128), np.ascontiguousarray(sdn),
            np.ascontiguousarray(sup))


def _build():
    import concourse.mybir as mybir
    import concourse.tile as tile
    from concourse import bacc

    F32 = mybir.dt.float32
    BF16 = mybir.dt.bfloat16
    I16 = mybir.dt.int16
    Exp = mybir.ActivationFunctionType.Exp
    ActCopy = mybir.ActivationFunctionType.Copy
    MULT = mybir.AluOpType.mult
    ADD = mybir.AluOpType.add
    DIV = mybir.AluOpType.divide

    nc = bacc.Bacc("TRN2", target_bir_lowering=False, debug=False,
                   num_devices=N_CORES)
    xt_d = nc.dram_tensor("xt", [HD, S], BF16, kind="ExternalInput")
    wq_d = nc.dram_tensor("wqkv", [HD, 3 * HG * D], BF16, kind="ExternalInput")
    ow_d = nc.dram_tensor("owt", [HG * D, HD], BF16, kind="ExternalInput")
    cos_d = nc.dram_tensor("cos", [128, S], BF16, kind="ExternalInput")
    sdn_d = nc.dram_tensor("sdn", [128, S], BF16, kind="ExternalInput")
    sup_d = nc.dram_tensor("sup", [128, S], BF16, kind="ExternalInput")
    out_d = nc.dram_tensor("out", [S, HD], F32, kind="ExternalOutput")

    SB = S // 512   # 4 q/s blocks
    SC = S // 128   # 16 k-chunks
    DC = HD // 128  # 8 contraction chunks for qkv proj

    with tile.TileContext(nc) as tc:
        with (
            tc.tile_pool(name="persist", bufs=1) as persist,
            tc.tile_pool(name="w1", bufs=1) as w1p,
            tc.tile_pool(name="xts", bufs=2) as xtp,
            tc.tile_pool(name="rope", bufs=3) as ropep,
            tc.tile_pool(name="ps", bufs=2, space="PSUM") as psp,
            tc.tile_pool(name="qsd", bufs=2, space="PSUM") as qsdp,
            tc.tile_pool(name="acc", bufs=2, space="PSUM") as accp,
            tc.tile_pool(name="es", bufs=2) as esp,
            tc.tile_pool(name="otq", bufs=2) as otqp,
            tc.tile_pool(name="rc", bufs=4) as rcp,
            tc.tile_pool(name="og", bufs=2) as ogp,
        ):
            # post-RoPE q (fc 0-3) / k (fc 4-7), [d(2 heads), s] bf16
            qk_sb = [persist.tile([128, S], BF16, tag=f"qk{i}", name=f"qk{i}")
                     for i in range(8)]
            # v^T + ones column: [s, chunk, head, d+1]
            v_sb = persist.tile([128, SC, HG, D + 1], BF16, tag="v")
            # o_proj lhsT: [d(head pair), s] bf16
            otT = [persist.tile([128, S], BF16, tag=f"ot{i}", name=f"ot{i}")
                   for i in range(4)]
            ow_sb = persist.tile([128, 4, HD], BF16, tag="ow")

            nc.gpsimd.memset(v_sb[:, :, :, D:D + 1], 1.0)  # softmax denom col

            # ---------------- phase 1: qkv projection + RoPE ----------------
            cos_sb = w1p.tile([128, S], BF16, tag="cos")
            # sina pre-shifted by +-32 partitions so the rotate-half muls
            # read both SBUF operands at EQUAL base partitions (hw rule)
            sdn_sb = w1p.tile([128, S], BF16, tag="sdn")
            sup_sb = w1p.tile([128, S], BF16, tag="sup")
            wq_sb = w1p.tile([128, DC, 3 * HG * D], BF16, tag="wq")
            wq_r = wq_d[:].rearrange("(a p) f -> p a f", p=128)
            xt_r = xt_d[:].rearrange("(a p) s -> p a s", p=128)

            xts = []
            for sb in range(SB):
                xts.append(xtp.tile([128, DC, 512], BF16, tag="xts",
                                    name=f"xts{sb}"))
            # DMA order: first s-block + k weights first (dc-halved) so the
            # first K-proj accumulation can start as early as possible.
            nc.sync.dma_start(xts[0][:, 0:4, :], xt_r[:, 0:4, 0:512])
            nc.sync.dma_start(wq_sb[:, 0:4, 512:1024], wq_r[:, 0:4, 512:1024])
            nc.sync.dma_start(xts[0][:, 4:8, :], xt_r[:, 4:8, 0:512])
            nc.sync.dma_start(wq_sb[:, 4:8, 512:1024], wq_r[:, 4:8, 512:1024])
            nc.sync.dma_start(wq_sb[:, :, 1024:1536], wq_r[:, :, 1024:1536])
            nc.sync.dma_start(sdn_sb[:], sdn_d[:])
            nc.sync.dma_start(sup_sb[:], sup_d[:])
            nc.sync.dma_start(cos_sb[:], cos_d[:])
            nc.sync.dma_start(wq_sb[:, :, 0:512], wq_r[:, :, 0:512])
            nc.sync.dma_start(
                ow_sb[:], ow_d[:].rearrange("(a p) f -> p a f", p=128)
            )

            def rope(ps, fc, ssl):
                """ps [128(d of 2 heads), 512] f32 psum -> qk_sb[fc] bf16.

                GPSIMD can't touch PSUM on real hw, so ACT (idle in
                phase 1) drains ps to SBUF; rotate-half + cos muls then
                run on Pool, the final add on DVE; sign folded into sina."""
                psb = ropep.tile([128, 512], F32, tag="psb")
                t = ropep.tile([128, 512], F32, tag="t")
                u = ropep.tile([128, 512], F32, tag="u")
                nc.scalar.activation(psb[:], ps[:], ActCopy)
                nc.gpsimd.tensor_mul(t[0:32, :], psb[32:64, :],
                                     sdn_sb[32:64, ssl])
                nc.gpsimd.tensor_mul(t[32:64, :], psb[0:32, :],
                                     sup_sb[0:32, ssl])
                nc.gpsimd.tensor_mul(t[64:96, :], psb[96:128, :],
                                     sdn_sb[96:128, ssl])
                nc.gpsimd.tensor_mul(t[96:128, :], psb[64:96, :],
                                     sup_sb[64:96, ssl])
                nc.gpsimd.tensor_mul(u[:], psb[:], cos_sb[:, ssl])
                nc.vector.tensor_add(qk_sb[fc][:, ssl], u[:], t[:])

            for sb in range(SB):
                ssl = slice(sb * 512, (sb + 1) * 512)
                if sb + 1 < SB:
                    nc.sync.dma_start(
                        xts[sb + 1][:],
                        xt_r[:, :, (sb + 1) * 512:(sb + 2) * 512],
                    )
                # K proj (fc 4-7) first so attention can start earliest.
                for fc in (4, 5, 6, 7):
                    ps = psp.tile([128, 1024], F32, tag="ps", name="ps")[:, 0:512]
                    for dc in range(DC):
                        nc.tensor.matmul(
                            ps[:],
                            wq_sb[:, dc, fc * 128:(fc + 1) * 128],
                            xts[sb][:, dc, :],
                            start=(dc == 0), stop=(dc == DC - 1),
                        )
                    rope(ps, fc, ssl)
                # V proj: out [s(128), vcols(512)]
                for s4 in range(4):
                    sc = sb * 4 + s4
                    ps = psp.tile([128, 1024], F32, tag="ps", name="ps")[:, 0:512]
                    for dc in range(DC):
                        nc.tensor.matmul(
                            ps[:],
                            xts[sb][:, dc, s4 * 128:(s4 + 1) * 128],
                            wq_sb[:, dc, 1024:1536],
                            start=(dc == 0), stop=(dc == DC - 1),
                        )
                    nc.scalar.activation(
                        v_sb[:, sc, :, 0:D],
                        ps[:].rearrange("p (h d) -> p h d", d=D),
                        ActCopy,
                    )
                # Q proj (fc 0-3)
                for fc in (0, 1, 2, 3):
                    ps = psp.tile([128, 1024], F32, tag="ps", name="ps")[:, 0:512]
                    for dc in range(DC):
                        nc.tensor.matmul(
                            ps[:],
                            wq_sb[:, dc, fc * 128:(fc + 1) * 128],
                            xts[sb][:, dc, :],
                            start=(dc == 0), stop=(dc == DC - 1),
                        )
                    rope(ps, fc, ssl)

            # ---------------- phase 2: attention + o_proj ----------------
            es_tiles = {}
            otp_tiles = {}

            # exp split: DVE Schraudolph singles at chunks {0,3,6,9,12,15}
            # (6/16 approximated), ACT table-exp on the remaining chunk
            # pairs (big ops amortize ACT's fixed overhead).  GPSIMD can't
            # read PSUM, so Pool gets no exp work.  Emission alternates
            # D,A,D,A,... so both engines stay fed and the two PSUM
            # rotations (qsd singles, ps pairs) are each deep enough.
            def emit_qk_exp(qb, h):
                qt, kt = qk_sb[h // 2], qk_sb[4 + h // 2]
                hp = (h % 2) * 64
                qsl = slice(qb * 512, (qb + 1) * 512)
                es = esp.tile([128, SC, 512], BF16, tag="es")
                es_tiles[(qb, h)] = es

                def qk_mm(qs_slice, c):
                    nc.tensor.matmul(
                        qs_slice,
                        kt[hp:hp + 64, c * 128:(c + 1) * 128],
                        qt[hp:hp + 64, qsl],
                        start=True, stop=True,
                    )

                for g in range(6):
                    cd = 3 * g           # 0, 3, 6, 9, 12, 15
                    qs = qsdp.tile([128, 512], F32, tag="qsd", name="qsd")
                    qk_mm(qs[:], cd)
                    nc.vector.tensor_scalar(
                        es[:, cd, :].bitcast(I16), qs[:],
                        A16 * 0.125, B16, MULT, ADD)
                    if g < 5:
                        ca = 3 * g + 1   # pairs (1,2),(4,5),(7,8),(10,11),(13,14)
                        qp = psp.tile([128, 1024], F32, tag="ps", name="qp")
                        qk_mm(qp[:, 0:512], ca)
                        qk_mm(qp[:, 512:1024], ca + 1)
                        dst2 = es[:, ca:ca + 2, :].rearrange("p a b -> p (a b)")
                        nc.scalar.activation(dst2, qp[:], Exp, scale=0.125)

            def emit_pv(qb, h, oproj_per_qsub=False):
                """pv[qpos, d+1] = sum_c es[c]^T @ v[c]; normalize with
                the ones-column denominator; DMA-transpose head pairs
                into o_proj lhsT layout."""
                es = es_tiles.pop((qb, h))
                hp2 = (h % 2) * 64
                if h % 2 == 0:
                    otp_tiles[qb] = otqp.tile([128, 4, 128], BF16,
                                              tag="otp", name=f"otp{qb}_{h}")
                otp = otp_tiles[qb]
                for qsub in range(4):
                    pvt = accp.tile([128, 512], F32, tag="acc", name="pvt")
                    qq = slice(qsub * 128, (qsub + 1) * 128)
                    for c in range(SC):
                        nc.tensor.matmul(
                            pvt[:, 0:D + 1],
                            es[:, c, qq],
                            v_sb[:, c, h, :],
                            start=(c == 0), stop=(c == SC - 1),
                        )
                    # normalize by the ones-column denominator: reciprocal
                    # to SBUF then per-partition scalar mult (DVE has no
                    # divide ISA op), PSUM -> bf16 SBUF
                    rc = rcp.tile([128, 1], F32, tag="rc")
                    nc.vector.reciprocal(rc[:], pvt[:, D:D + 1])
                    nc.vector.tensor_scalar(
                        otp[:, qsub, hp2:hp2 + D], pvt[:, 0:D],
                        rc[:], None, MULT)
                    if h % 2 == 1:
                        nc.sync.dma_start_transpose(
                            otT[h // 2][:, qb * 512 + qsub * 128:
                                        qb * 512 + (qsub + 1) * 128],
                            otp[:, qsub, :],
                        )
                        # trail by one qsub so the o_proj lhsT transpose
                        # latency hides behind the next PV group
                        if oproj_per_qsub and qsub > 0:
                            emit_oproj_s4(qb, qsub - 1)
                if oproj_per_qsub:
                    emit_oproj_s4(qb, 3)

            def emit_oproj_s4(qb, s4):
                sc = qb * 4 + s4
                og = ogp.tile([128, HD], F32, tag="og", name=f"og{sc}")
                for jb in range(2):
                    po = accp.tile([128, 512], F32, tag="acc", name="po")
                    for oc in range(4):
                        nc.tensor.matmul(
                            po[:],
                            otT[oc][:, sc * 128:(sc + 1) * 128],
                            ow_sb[:, oc, jb * 512:(jb + 1) * 512],
                            start=(oc == 0), stop=(oc == 3),
                        )
                    # ACT idles during o_proj windows (no exps there), so
                    # it takes the PSUM drain instead of DVE
                    nc.scalar.activation(
                        og[:, jb * 512:(jb + 1) * 512], po[:], ActCopy)
                    nc.sync.dma_start(
                        out_d[sc * 128:(sc + 1) * 128,
                              jb * 512:(jb + 1) * 512],
                        og[:, jb * 512:(jb + 1) * 512])

            def emit_oproj(qb):
                for s4 in range(4):
                    emit_oproj_s4(qb, s4)

            # one-block software pipeline: PV of block i runs while
            # exp of block i+1 is in flight.  PV (and its DVE
            # recip/normalize) is emitted BEFORE block i+1's exps so
            # the normalize isn't queued behind them on DVE.
            prev = None
            for qb in range(SB):
                for h in range(HG):
                    if prev is not None:
                        emit_pv(*prev)
                    emit_qk_exp(qb, h)
                    if prev is not None and prev[1] == HG - 1:
                        emit_oproj(prev[0])
                    prev = (qb, h)
            emit_pv(*prev, oproj_per_qsub=True)

    nc.compile()
    return nc


def kernel(hidden_states, qkv_w, o_w):
    global last_results
    from concourse.bass_utils import run_bass_kernel_spmd

    hidden_states = np.asarray(hidden_states, dtype=np.float32)
    qkv_w = np.asarray(qkv_w, dtype=np.float32)
    o_w = np.asarray(o_w, dtype=np.float32)

    cos128, sdn128, sup128 = _rope_tables()
    nc = _build()

    bf16 = ml_dtypes.bfloat16
    in_maps = []
    for core in range(N_CORES):
        b, g = core // G, core % G
        heads = range(g * HG, (g + 1) * HG)
        rows = np.concatenate(
              [np.arange(h * D, (h + 1) * D) for h in heads])
        wsel = np.concatenate(
              [qkv_w[off + rows] for off in (0, HD, 2 * HD)], axis=0)  # [1536,1024]
        in_maps.append({
              "xt": np.ascontiguousarray(hidden_states[b].T).astype(bf16),
              "wqkv": np.ascontiguousarray(wsel.T).astype(bf16),
              "owt": np.ascontiguousarray(o_w[:, rows].T).astype(bf16),
              "cos": cos128,
              "sdn": sdn128,
              "sup": sup128,
        })

    trace = bool(int(os.environ.get("KERNEL_TRACE", "0")))
    try:
        last_results = run_bass_kernel_spmd(
            nc, in_maps, core_ids=list(range(N_CORES)), trace=trace)
    except ModuleNotFoundError:
        # axon NTFF hook unavailable in this container; run without trace
        last_results = run_bass_kernel_spmd(
            nc, in_maps, core_ids=list(range(N_CORES)), trace=False)

    out = np.empty((B, S, HD), dtype=np.float32)
    for b in range(B):
        out[b] = last_results.results[b * G]["out"]
        for g in range(1, G):
              out[b] += last_results.results[b * G + g]["out"]
    return out


# revision 35
# speedup vs baseline: 1.0027x; 1.0027x over previous
"""KindredAttention on 8 trn2 NeuronCores.

Sharding: core(b, g) = b*2 + g for batch b in 0..3, head-group g in 0..1
(8 heads per group). Data-parallel over batch, tensor-parallel over heads
(qkv column-split, o_proj row-split; host sums the two o_proj partials).

Per-core layouts (host pre-transposes; all inputs bf16):
  xt   [1024, 2048] bf16 : hidden[b].T                  (d-major)
  wqkv [1024, 1536] bf16 : qkv_w rows for this group, transposed.
                           cols = [q(8 heads x 64) | k(...) | v(...)]
  owt  [512, 1024]  bf16 : o_w[:, group cols].T
  cos/sina [128, 2048] bf16 : RoPE tables (2 heads stacked, sign-folded sin)
  out  [2048, 1024] f32  : partial o_proj output (host adds g=0 + g=1)

Engine budget per core (cost model): PE ~275us is the floor
(proj 82 + QK 109 + PV 55 + o_proj 27us).  The 33.5M-score exp is
split: ACT table-exp on 10/16 k-chunks (as [128,1024] pair ops that
amortize ACT's fixed overhead), DVE Schraudolph int16-exp (bitcast as
bf16) on 6/16.  GPSIMD/Pool cannot touch PSUM on real hw, so ACT
drains qkv-proj PSUM to SBUF (idle in phase 1) and Pool runs the RoPE
rotate/cos muls from there (sina tables pre-shifted +-32 partitions:
both SBUF operands of a TensorTensor must share a base partition);
the combine-add is DVE.  PV is es-stationary (lhsT = scores, moving =
v, 65-wide output) which halves its PE cost vs v-stationary; the
softmax denominator rides a ones-column in v; normalize is
reciprocal + per-partition tensor_scalar into [qpos, d] bf16, and the
o_proj lhsT layout [d(head pair), qpos] comes from a SBUF->SBUF DMA
transpose (XBAR).  o_proj drains ride ACT (idle in o_proj windows).
A one-block software pipeline defers each block's PV behind the next
block's QK/exp; o_proj is emitted per completed q-block.  A single
PSUM pool serves qkv-proj and the ACT exp pairs, so there is no pool
barrier between phases: banks = 2x[128,1024] pairs + 2x[128,512]
Schraudolph singles + 2x[128,512] shared pv/o_proj accumulators.
"""

import os

import ml_dtypes
import numpy as np

H = 16
D = 64
BASE = 10000.0
B, S, HD = 4, 2048, 1024
G = 2          # head groups (tensor parallel)
HG = H // G    # heads per group = 8
N_CORES = 8

# Schraudolph exp in bf16-bit space: exp(x) ~= bitcast_bf16(int16(
#   x * (2^7/ln2) + (127*2^7 - C16))).  C16 calibrated for floor();
# ~1.8% rms, ~4.2% max rel err on the scores this problem produces.
A16 = 128.0 / float(np.log(2.0))
C16 = 7.0
B16 = 127.0 * 128.0 - C16

last_results = None  # stash for test.py (exec_time_ns etc.)


def _rope_tables():
    inv_freq = 1.0 / (BASE ** (np.arange(0, D, 2, dtype=np.float32) / D))
    t = np.arange(S, dtype=np.float32)
    freqs = np.outer(t, inv_freq)                       # [S, 32]
    emb = np.concatenate([freqs, freqs], -1)            # [S, 64]
    cos = np.cos(emb).T.astype(np.float32)              # [64, S]
    sin = np.sin(emb).T.astype(np.float32)
    sina = sin.copy()
    sina[:32] = -sina[:32]                              # rotate_half sign fold
    cos128 = np.tile(cos, (2, 1)).astype(ml_dtypes.bfloat16)    # [128, S]
    sina128 = np.tile(sina, (2, 1)).astype(ml_dtypes.bfloat16)
    sdn = np.roll(sina128, 32, axis=0)    # row r holds sina[r-32]
    sup = np.roll(sina128, -32, axis=0)   # row r holds sina[r+32]
    return (np.ascontiguousarray(cos128), np.ascontiguousarray(sdn),
            np.ascontiguousarray(sup))


def _build():
    import concourse.mybir as mybir
    import concourse.tile as tile
    from concourse import bacc

    F32 = mybir.dt.float32
    BF16 = mybir.dt.bfloat16
    I16 = mybir.dt.int16
    Exp = mybir.ActivationFunctionType.Exp
    ActCopy = mybir.ActivationFunctionType.Copy
    MULT = mybir.AluOpType.mult
    ADD = mybir.AluOpType.add
    DIV = mybir.AluOpType.divide

    nc = bacc.Bacc("TRN2", target_bir_lowering=False, debug=False,
                   num_devices=N_CORES)
    xt_d = nc.dram_tensor("xt", [HD, S], BF16, kind="ExternalInput")
    wq_d = nc.dram_tensor("wqkv", [HD, 3 * HG * D], BF16, kind="ExternalInput")
    ow_d = nc.dram_tensor("owt", [HG * D, HD], BF16, kind="ExternalInput")
    cos_d = nc.dram_tensor("cos", [128, S], BF16, kind="ExternalInput")
    sdn_d = nc.dram_tensor("sdn", [128, S], BF16, kind="ExternalInput")
    sup_d = nc.dram_tensor("sup", [128, S], BF16, kind="ExternalInput")
    out_d = nc.dram_tensor("out", [S, HD], F32, kind="ExternalOutput")

    SB = S // 512   # 4 q/s blocks
    SC = S // 128   # 16 k-chunks
    DC = HD // 128  # 8 contraction chunks for qkv proj

    with tile.TileContext(nc) as tc:
        with (
            tc.tile_pool(name="persist", bufs=1) as persist,
            tc.tile_pool(name="w1", bufs=1) as w1p,
            tc.tile_pool(name="xts", bufs=2) as xtp,
            tc.tile_pool(name="rope", bufs=3) as ropep,
            tc.tile_pool(name="ps", bufs=2, space="PSUM") as psp,
            tc.tile_pool(name="qsd", bufs=2, space="PSUM") as qsdp,
            tc.tile_pool(name="acc", bufs=2, space="PSUM") as accp,
            tc.tile_pool(name="es", bufs=2) as esp,
            tc.tile_pool(name="otq", bufs=2) as otqp,
            tc.tile_pool(name="rc", bufs=4) as rcp,
            tc.tile_pool(name="og", bufs=2) as ogp,
        ):
            # post-RoPE q (fc 0-3) / k (fc 4-7), [d(2 heads), s] bf16
            qk_sb = [persist.tile([128, S], BF16, tag=f"qk{i}", name=f"qk{i}")
                     for i in range(8)]
            # v^T + ones column: [s, chunk, head, d+1]
            v_sb = persist.tile([128, SC, HG, D + 1], BF16, tag="v")
            # o_proj lhsT: [d(head pair), s] bf16
            otT = [persist.tile([128, S], BF16, tag=f"ot{i}", name=f"ot{i}")
                   for i in range(4)]
            ow_sb = persist.tile([128, 4, HD], BF16, tag="ow")

            nc.gpsimd.memset(v_sb[:, :, :, D:D + 1], 1.0)  # softmax denom col

            # ---------------- phase 1: qkv projection + RoPE ----------------
            cos_sb = w1p.tile([128, S], BF16, tag="cos")
            # sina pre-shifted by +-32 partitions so the rotate-half muls
            # read both SBUF operands at EQUAL base partitions (hw rule)
            sdn_sb = w1p.tile([128, S], BF16, tag="sdn")
            sup_sb = w1p.tile([128, S], BF16, tag="sup")
            wq_sb = w1p.tile([128, DC, 3 * HG * D], BF16, tag="wq")
            wq_r = wq_d[:].rearrange("(a p) f -> p a f", p=128)
            xt_r = xt_d[:].rearrange("(a p) s -> p a s", p=128)

            xts = []
            for sb in range(SB):
                xts.append(xtp.tile([128, DC, 512], BF16, tag="xts",
                                    name=f"xts{sb}"))
            # DMA order: first s-block + k weights first (dc-halved) so the
            # first K-proj accumulation can start as early as possible.
            nc.sync.dma_start(xts[0][:, 0:4, :], xt_r[:, 0:4, 0:512])
            nc.sync.dma_start(wq_sb[:, :, 512:640], wq_r[:, :, 512:640])
            nc.sync.dma_start(xts[0][:, 4:8, :], xt_r[:, 4:8, 0:512])
            nc.sync.dma_start(wq_sb[:, :, 640:768], wq_r[:, :, 640:768])
            nc.sync.dma_start(wq_sb[:, :, 768:1024], wq_r[:, :, 768:1024])
            nc.sync.dma_start(wq_sb[:, :, 1024:1536], wq_r[:, :, 1024:1536])
            nc.sync.dma_start(sdn_sb[:], sdn_d[:])
            nc.sync.dma_start(sup_sb[:], sup_d[:])
            nc.sync.dma_start(cos_sb[:], cos_d[:])
            nc.sync.dma_start(wq_sb[:, :, 0:512], wq_r[:, :, 0:512])
            nc.sync.dma_start(
                ow_sb[:], ow_d[:].rearrange("(a p) f -> p a f", p=128)
            )

            def rope(ps, fc, ssl):
                """ps [128(d of 2 heads), 512] f32 psum -> qk_sb[fc] bf16.

                GPSIMD can't touch PSUM on real hw, so ACT (idle in
                phase 1) drains ps to SBUF; rotate-half + cos muls then
                run on Pool, the final add on DVE; sign folded into sina."""
                psb = ropep.tile([128, 512], F32, tag="psb")
                t = ropep.tile([128, 512], F32, tag="t")
                u = ropep.tile([128, 512], F32, tag="u")
                nc.scalar.activation(psb[:], ps[:], ActCopy)
                nc.gpsimd.tensor_mul(t[0:32, :], psb[32:64, :],
                                     sdn_sb[32:64, ssl])
                nc.gpsimd.tensor_mul(t[32:64, :], psb[0:32, :],
                                     sup_sb[0:32, ssl])
                nc.gpsimd.tensor_mul(t[64:96, :], psb[96:128, :],
                                     sdn_sb[96:128, ssl])
                nc.gpsimd.tensor_mul(t[96:128, :], psb[64:96, :],
                                     sup_sb[64:96, ssl])
                nc.gpsimd.tensor_mul(u[:], psb[:], cos_sb[:, ssl])
                nc.vector.tensor_add(qk_sb[fc][:, ssl], u[:], t[:])

            for sb in range(SB):
                ssl = slice(sb * 512, (sb + 1) * 512)
                if sb + 1 < SB:
                    nc.sync.dma_start(
                        xts[sb + 1][:],
                        xt_r[:, :, (sb + 1) * 512:(sb + 2) * 512],
                    )
                # K proj (fc 4-7) first so attention can start earliest.
                for fc in (4, 5, 6, 7):
                    if fc % 2 == 0:
                        ps = psp.tile([128, 1024], F32, tag="ps",
                                      name="ps")[:, 0:512]
                    else:
                        ps = qsdp.tile([128, 512], F32, tag="qsd", name="qsd")
                    for dc in range(DC):
                        nc.tensor.matmul(
                            ps[:],
                            wq_sb[:, dc, fc * 128:(fc + 1) * 128],
                            xts[sb][:, dc, :],
                            start=(dc == 0), stop=(dc == DC - 1),
                        )
                    rope(ps, fc, ssl)
                # V proj: out [s(128), vcols(512)]
                for s4 in range(4):
                    sc = sb * 4 + s4
                    if s4 % 2 == 0:
                        ps = psp.tile([128, 1024], F32, tag="ps",
                                      name="ps")[:, 0:512]
                    else:
                        ps = qsdp.tile([128, 512], F32, tag="qsd", name="qsd")
                    for dc in range(DC):
                        nc.tensor.matmul(
                            ps[:],
                            xts[sb][:, dc, s4 * 128:(s4 + 1) * 128],
                            wq_sb[:, dc, 1024:1536],
                            start=(dc == 0), stop=(dc == DC - 1),
                        )
                    nc.scalar.activation(
                        v_sb[:, sc, :, 0:D],
                        ps[:].rearrange("p (h d) -> p h d", d=D),
                        ActCopy,
                    )
                # Q proj (fc 0-3)
                for fc in (0, 1, 2, 3):
                    if fc % 2 == 0:
                        ps = psp.tile([128, 1024], F32, tag="ps",
                                      name="ps")[:, 0:512]
                    else:
                        ps = qsdp.tile([128, 512], F32, tag="qsd", name="qsd")
                    for dc in range(DC):
                        nc.tensor.matmul(
                            ps[:],
                            wq_sb[:, dc, fc * 128:(fc + 1) * 128],
                            xts[sb][:, dc, :],
                            start=(dc == 0), stop=(dc == DC - 1),
                        )
                    rope(ps, fc, ssl)

            # ---------------- phase 2: attention + o_proj ----------------
            es_tiles = {}
            otp_tiles = {}

            # exp split: DVE Schraudolph singles at chunks {0,3,6,9,12,15}
            # (6/16 approximated), ACT table-exp on the remaining chunk
            # pairs (big ops amortize ACT's fixed overhead).  GPSIMD can't
            # read PSUM, so Pool gets no exp work.  Emission alternates
            # D,A,D,A,... so both engines stay fed and the two PSUM
            # rotations (qsd singles, ps pairs) are each deep enough.
            def emit_qk_exp(qb, h):
                qt, kt = qk_sb[h // 2], qk_sb[4 + h // 2]
                hp = (h % 2) * 64
                qsl = slice(qb * 512, (qb + 1) * 512)
                es = esp.tile([128, SC, 512], BF16, tag="es")
                es_tiles[(qb, h)] = es

                def qk_mm(qs_slice, c):
                    nc.tensor.matmul(
                        qs_slice,
                        kt[hp:hp + 64, c * 128:(c + 1) * 128],
                        qt[hp:hp + 64, qsl],
                        start=True, stop=True,
                    )

                for g in range(6):
                    cd = 3 * g           # 0, 3, 6, 9, 12, 15
                    qs = qsdp.tile([128, 512], F32, tag="qsd", name="qsd")
                    qk_mm(qs[:], cd)
                    nc.vector.tensor_scalar(
                        es[:, cd, :].bitcast(I16), qs[:],
                        A16 * 0.125, B16, MULT, ADD)
                    if g < 5:
                        ca = 3 * g + 1   # pairs (1,2),(4,5),(7,8),(10,11),(13,14)
                        qp = psp.tile([128, 1024], F32, tag="ps", name="qp")
                        qk_mm(qp[:, 0:512], ca)
                        qk_mm(qp[:, 512:1024], ca + 1)
                        dst2 = es[:, ca:ca + 2, :].rearrange("p a b -> p (a b)")
                        nc.scalar.activation(dst2, qp[:], Exp, scale=0.125)

            def emit_pv(qb, h, oproj_per_qsub=False):
                """pv[qpos, d+1] = sum_c es[c]^T @ v[c]; normalize with
                the ones-column denominator; DMA-transpose head pairs
                into o_proj lhsT layout."""
                es = es_tiles.pop((qb, h))
                hp2 = (h % 2) * 64
                if h % 2 == 0:
                    otp_tiles[qb] = otqp.tile([128, 4, 128], BF16,
                                              tag="otp", name=f"otp{qb}_{h}")
                otp = otp_tiles[qb]
                for qsub in range(4):
                    pvt = accp.tile([128, 512], F32, tag="acc", name="pvt")
                    qq = slice(qsub * 128, (qsub + 1) * 128)
                    for c in range(SC):
                        nc.tensor.matmul(
                            pvt[:, 0:D + 1],
                            es[:, c, qq],
                            v_sb[:, c, h, :],
                            start=(c == 0), stop=(c == SC - 1),
                        )
                    # normalize by the ones-column denominator: reciprocal
                    # to SBUF then per-partition scalar mult (DVE has no
                    # divide ISA op), PSUM -> bf16 SBUF
                    rc = rcp.tile([128, 1], F32, tag="rc")
                    nc.vector.reciprocal(rc[:], pvt[:, D:D + 1])
                    nc.vector.tensor_scalar(
                        otp[:, qsub, hp2:hp2 + D], pvt[:, 0:D],
                        rc[:], None, MULT)
                    if h % 2 == 1:
                        nc.sync.dma_start_transpose(
                            otT[h // 2][:, qb * 512 + qsub * 128:
                                        qb * 512 + (qsub + 1) * 128],
                            otp[:, qsub, :],
                        )
                        # trail by one qsub so the o_proj lhsT transpose
                        # latency hides behind the next PV group
                        if oproj_per_qsub and qsub > 0:
                            emit_oproj_s4(qb, qsub - 1)
                if oproj_per_qsub:
                    emit_oproj_s4(qb, 3)

            def emit_oproj_s4(qb, s4):
                sc = qb * 4 + s4
                og = ogp.tile([128, HD], F32, tag="og", name=f"og{sc}")
                for jb in range(2):
                    po = accp.tile([128, 512], F32, tag="acc", name="po")
                    for oc in range(4):
                        nc.tensor.matmul(
                            po[:],
                            otT[oc][:, sc * 128:(sc + 1) * 128],
                            ow_sb[:, oc, jb * 512:(jb + 1) * 512],
                            start=(oc == 0), stop=(oc == 3),
                        )
                    # ACT idles during o_proj windows (no exps there), so
                    # it takes the PSUM drain instead of DVE
                    nc.scalar.activation(
                        og[:, jb * 512:(jb + 1) * 512], po[:], ActCopy)
                    nc.sync.dma_start(
                        out_d[sc * 128:(sc + 1) * 128,
                              jb * 512:(jb + 1) * 512],
                        og[:, jb * 512:(jb + 1) * 512])

            def emit_oproj(qb):
                for s4 in range(4):
                    emit_oproj_s4(qb, s4)

            # one-block software pipeline: PV of block i runs while
            # exp of block i+1 is in flight.  PV (and its DVE
            # recip/normalize) is emitted BEFORE block i+1's exps so
            # the normalize isn't queued behind them on DVE.
            prev = None
            for qb in range(SB):
                for h in range(HG):
                    if prev is not None:
                        emit_pv(*prev)
                    emit_qk_exp(qb, h)
                    if prev is not None and prev[1] == HG - 1:
                        emit_oproj(prev[0])
                    prev = (qb, h)
            emit_pv(*prev, oproj_per_qsub=True)

    nc.compile()
    return nc


def kernel(hidden_states, qkv_w, o_w):
    global last_results
    from concourse.bass_utils import run_bass_kernel_spmd

    hidden_states = np.asarray(hidden_states, dtype=np.float32)
    qkv_w = np.asarray(qkv_w, dtype=np.float32)
    o_w = np.asarray(o_w, dtype=np.float32)

    cos128, sdn128, sup128 = _rope_tables()
    nc = _build()

    bf16 = ml_dtypes.bfloat16
    in_maps = []
    for core in range(N_CORES):
        b, g = core // G, core % G
        heads = range(g * HG, (g + 1) * HG)
        rows = np.concatenate(
              [np.arange(h * D, (h + 1) * D) for h in heads])
        wsel = np.concatenate(
              [qkv_w[off + rows] for off in (0, HD, 2 * HD)], axis=0)  # [1536,1024]
        in_maps.append({
              "xt": np.ascontiguousarray(hidden_states[b].T).astype(bf16),
              "wqkv": np.ascontiguousarray(wsel.T).astype(bf16),
              "owt": np.ascontiguousarray(o_w[:, rows].T).astype(bf16),
              "cos": cos128,
              "sdn": sdn128,
              "sup": sup128,
        })

    trace = bool(int(os.environ.get("KERNEL_TRACE", "0")))
    try:
        last_results = run_bass_kernel_spmd(
            nc, in_maps, core_ids=list(range(N_CORES)), trace=trace)
    except ModuleNotFoundError:
        # axon NTFF hook unavailable in this container; run without trace
        last_results = run_bass_kernel_spmd(
            nc, in_maps, core_ids=list(range(N_CORES)), trace=False)

    out = np.empty((B, S, HD), dtype=np.float32)
    for b in range(B):
        out[b] = last_results.results[b * G]["out"]
        for g in range(1, G):
              out[b] += last_results.results[b * G + g]["out"]
    return out


# revision 39
# speedup vs baseline: 1.0115x; 1.0087x over previous
"""KindredAttention on 8 trn2 NeuronCores.

Sharding: core(b, g) = b*2 + g for batch b in 0..3, head-group g in 0..1
(8 heads per group). Data-parallel over batch, tensor-parallel over heads
(qkv column-split, o_proj row-split; host sums the two o_proj partials).

Per-core layouts (host pre-transposes; all inputs bf16):
  xt   [1024, 2048] bf16 : hidden[b].T                  (d-major)
  wqkv [1024, 1536] bf16 : qkv_w rows for this group, transposed.
                           cols = [q(8 heads x 64) | k(...) | v(...)]
  owt  [512, 1024]  bf16 : o_w[:, group cols].T
  cos/sina [128, 2048] bf16 : RoPE tables (2 heads stacked, sign-folded sin)
  out  [2048, 1024] f32  : partial o_proj output (host adds g=0 + g=1)

Engine budget per core (cost model): PE ~275us is the floor
(proj 82 + QK 109 + PV 55 + o_proj 27us).  The 33.5M-score exp is
split: ACT table-exp on 10/16 k-chunks (as [128,1024] pair ops that
amortize ACT's fixed overhead), DVE Schraudolph int16-exp (bitcast as
bf16) on 6/16.  GPSIMD/Pool cannot touch PSUM on real hw, so ACT
drains qkv-proj PSUM to SBUF (idle in phase 1) and Pool runs the RoPE
rotate/cos muls from there (sina tables pre-shifted +-32 partitions:
both SBUF operands of a TensorTensor must share a base partition);
the combine-add is DVE.  PV is es-stationary (lhsT = scores, moving =
v, 65-wide output) which halves its PE cost vs v-stationary; the
softmax denominator rides a ones-column in v; normalize is
reciprocal + per-partition tensor_scalar into [qpos, d] bf16, and the
o_proj lhsT layout [d(head pair), qpos] comes from a SBUF->SBUF DMA
transpose (XBAR).  o_proj drains ride ACT (idle in o_proj windows).
A one-block software pipeline defers each block's PV behind the next
block's QK/exp; o_proj is emitted per completed q-block.  A single
PSUM pool serves qkv-proj and the ACT exp pairs, so there is no pool
barrier between phases: banks = 2x[128,1024] pairs + 2x[128,512]
Schraudolph singles + 2x[128,512] shared pv/o_proj accumulators.
"""

import os

import ml_dtypes
import numpy as np

H = 16
D = 64
BASE = 10000.0
B, S, HD = 4, 2048, 1024
G = 2          # head groups (tensor parallel)
HG = H // G    # heads per group = 8
N_CORES = 8

# Schraudolph exp in bf16-bit space: exp(x) ~= bitcast_bf16(int16(
#   x * (2^7/ln2) + (127*2^7 - C16))).  C16 calibrated for floor();
# ~1.8% rms, ~4.2% max rel err on the scores this problem produces.
A16 = 128.0 / float(np.log(2.0))
C16 = 7.0
B16 = 127.0 * 128.0 - C16

last_results = None  # stash for test.py (exec_time_ns etc.)


def _rope_tables():
    inv_freq = 1.0 / (BASE ** (np.arange(0, D, 2, dtype=np.float32) / D))
    t = np.arange(S, dtype=np.float32)
    freqs = np.outer(t, inv_freq)                       # [S, 32]
    emb = np.concatenate([freqs, freqs], -1)            # [S, 64]
    cos = np.cos(emb).T.astype(np.float32)              # [64, S]
    sin = np.sin(emb).T.astype(np.float32)
    sina = sin.copy()
    sina[:32] = -sina[:32]                              # rotate_half sign fold
    cos128 = np.tile(cos, (2, 1)).astype(ml_dtypes.bfloat16)    # [128, S]
    sina128 = np.tile(sina, (2, 1)).astype(ml_dtypes.bfloat16)
    sdn = np.roll(sina128, 32, axis=0)    # row r holds sina[r-32]
    sup = np.roll(sina128, -32, axis=0)   # row r holds sina[r+32]
    return (np.ascontiguousarray(cos128), np.ascontiguousarray(sdn),
            np.ascontiguousarray(sup))


def _build():
    import concourse.mybir as mybir
    import concourse.tile as tile
    from concourse import bacc

    F32 = mybir.dt.float32
    BF16 = mybir.dt.bfloat16
    I16 = mybir.dt.int16
    Exp = mybir.ActivationFunctionType.Exp
    ActCopy = mybir.ActivationFunctionType.Copy
    MULT = mybir.AluOpType.mult
    ADD = mybir.AluOpType.add
    DIV = mybir.AluOpType.divide

    nc = bacc.Bacc("TRN2", target_bir_lowering=False, debug=False,
                   num_devices=N_CORES)
    xt_d = nc.dram_tensor("xt", [HD, S], BF16, kind="ExternalInput")
    wq_d = nc.dram_tensor("wqkv", [HD, 3 * HG * D], BF16, kind="ExternalInput")
    ow_d = nc.dram_tensor("owt", [HG * D, HD], BF16, kind="ExternalInput")
    cos_d = nc.dram_tensor("cos", [128, S], BF16, kind="ExternalInput")
    sdn_d = nc.dram_tensor("sdn", [128, S], BF16, kind="ExternalInput")
    sup_d = nc.dram_tensor("sup", [128, S], BF16, kind="ExternalInput")
    out_d = nc.dram_tensor("out", [S, HD], F32, kind="ExternalOutput")

    SB = S // 512   # 4 q/s blocks
    SC = S // 128   # 16 k-chunks
    DC = HD // 128  # 8 contraction chunks for qkv proj

    with tile.TileContext(nc) as tc:
        with (
            tc.tile_pool(name="persist", bufs=1) as persist,
            tc.tile_pool(name="w1", bufs=1) as w1p,
            tc.tile_pool(name="xts", bufs=2) as xtp,
            tc.tile_pool(name="rope", bufs=3) as ropep,
            tc.tile_pool(name="ps", bufs=2, space="PSUM") as psp,
            tc.tile_pool(name="qsd", bufs=2, space="PSUM") as qsdp,
            tc.tile_pool(name="acc", bufs=2, space="PSUM") as accp,
            tc.tile_pool(name="es", bufs=2) as esp,
            tc.tile_pool(name="otq", bufs=2) as otqp,
            tc.tile_pool(name="rc", bufs=4) as rcp,
            tc.tile_pool(name="og", bufs=2) as ogp,
        ):
            # post-RoPE q (fc 0-3) / k (fc 4-7), [d(2 heads), s] bf16
            qk_sb = [persist.tile([128, S], BF16, tag=f"qk{i}", name=f"qk{i}")
                     for i in range(8)]
            # v^T + ones column: [s, chunk, head, d+1]
            v_sb = persist.tile([128, SC, HG, D + 1], BF16, tag="v")
            # o_proj lhsT: [d(head pair), s] bf16
            otT = [persist.tile([128, S], BF16, tag=f"ot{i}", name=f"ot{i}")
                   for i in range(4)]
            ow_sb = persist.tile([128, 4, HD], BF16, tag="ow")

            nc.gpsimd.memset(v_sb[:, :, :, D:D + 1], 1.0)  # softmax denom col

            # PE clock warmup: the p-state model runs the first ~3us of
            # matmuls at 1/2 to 1/4 speed.  Burn the ramp on garbage
            # matmuls during the initial DMA wait so the real projection
            # starts at full clock (~1.8us saved).
            wmup = w1p.tile([128, 512], BF16, tag="wmup")
            nc.gpsimd.memset(wmup[:], 0.0)
            wps = accp.tile([128, 512], F32, tag="acc", name="wps")
            for _ in range(8):
                nc.tensor.matmul(wps[:], wmup[:, 0:128], wmup[:],
                                 start=True, stop=True)
            for _ in range(24):
                nc.tensor.matmul(wps[:, 0:64], wmup[:, 0:128], wmup[:, 0:64],
                                 start=True, stop=True)

            # ---------------- phase 1: qkv projection + RoPE ----------------
            cos_sb = w1p.tile([128, S], BF16, tag="cos")
            # sina pre-shifted by +-32 partitions so the rotate-half muls
            # read both SBUF operands at EQUAL base partitions (hw rule)
            sdn_sb = w1p.tile([128, S], BF16, tag="sdn")
            sup_sb = w1p.tile([128, S], BF16, tag="sup")
            wq_sb = w1p.tile([128, DC, 3 * HG * D], BF16, tag="wq")
            wq_r = wq_d[:].rearrange("(a p) f -> p a f", p=128)
            xt_r = xt_d[:].rearrange("(a p) s -> p a s", p=128)

            xts = []
            for sb in range(SB):
                xts.append(xtp.tile([128, DC, 512], BF16, tag="xts",
                                    name=f"xts{sb}"))
            # DMA order: first s-block + k weights first (dc-halved) so the
            # first K-proj accumulation can start as early as possible.
            nc.sync.dma_start(xts[0][:, 0:4, :], xt_r[:, 0:4, 0:512])
            nc.sync.dma_start(wq_sb[:, :, 512:768], wq_r[:, :, 512:768])
            nc.sync.dma_start(xts[0][:, 4:8, :], xt_r[:, 4:8, 0:512])
            nc.sync.dma_start(wq_sb[:, :, 768:1024], wq_r[:, :, 768:1024])
            nc.sync.dma_start(wq_sb[:, :, 1024:1536], wq_r[:, :, 1024:1536])
            nc.sync.dma_start(sdn_sb[:], sdn_d[:])
            nc.sync.dma_start(sup_sb[:], sup_d[:])
            nc.sync.dma_start(cos_sb[:], cos_d[:])
            nc.sync.dma_start(wq_sb[:, :, 0:512], wq_r[:, :, 0:512])
            nc.sync.dma_start(
                ow_sb[:], ow_d[:].rearrange("(a p) f -> p a f", p=128)
            )

            def rope(ps, fc, ssl):
                """ps [128(d of 2 heads), 512] f32 psum -> qk_sb[fc] bf16.

                GPSIMD can't touch PSUM on real hw, so ACT (idle in
                phase 1) drains ps to SBUF; rotate-half + cos muls then
                run on Pool, the final add on DVE; sign folded into sina."""
                psb = ropep.tile([128, 512], F32, tag="psb")
                t = ropep.tile([128, 512], F32, tag="t")
                u = ropep.tile([128, 512], F32, tag="u")
                nc.scalar.activation(psb[:], ps[:], ActCopy)
                nc.gpsimd.tensor_mul(t[0:32, :], psb[32:64, :],
                                     sdn_sb[32:64, ssl])
                nc.gpsimd.tensor_mul(t[32:64, :], psb[0:32, :],
                                     sup_sb[0:32, ssl])
                nc.gpsimd.tensor_mul(t[64:96, :], psb[96:128, :],
                                     sdn_sb[96:128, ssl])
                nc.gpsimd.tensor_mul(t[96:128, :], psb[64:96, :],
                                     sup_sb[64:96, ssl])
                nc.gpsimd.tensor_mul(u[:], psb[:], cos_sb[:, ssl])
                nc.vector.tensor_add(qk_sb[fc][:, ssl], u[:], t[:])

            for sb in range(SB):
                ssl = slice(sb * 512, (sb + 1) * 512)
                if sb + 1 < SB:
                    nc.sync.dma_start(
                        xts[sb + 1][:],
                        xt_r[:, :, (sb + 1) * 512:(sb + 2) * 512],
                    )
                # K proj (fc 4-7) first so attention can start earliest.
                for fc in (4, 5, 6, 7):
                    if fc % 2 == 0:
                        ps = psp.tile([128, 1024], F32, tag="ps",
                                      name="ps")[:, 0:512]
                    else:
                        ps = qsdp.tile([128, 512], F32, tag="qsd", name="qsd")
                    for dc in range(DC):
                        nc.tensor.matmul(
                            ps[:],
                            wq_sb[:, dc, fc * 128:(fc + 1) * 128],
                            xts[sb][:, dc, :],
                            start=(dc == 0), stop=(dc == DC - 1),
                        )
                    rope(ps, fc, ssl)
                # V proj: out [s(128), vcols(512)]
                for s4 in range(4):
                    sc = sb * 4 + s4
                    if s4 % 2 == 0:
                        ps = psp.tile([128, 1024], F32, tag="ps",
                                      name="ps")[:, 0:512]
                    else:
                        ps = qsdp.tile([128, 512], F32, tag="qsd", name="qsd")
                    for dc in range(DC):
                        nc.tensor.matmul(
                            ps[:],
                            xts[sb][:, dc, s4 * 128:(s4 + 1) * 128],
                            wq_sb[:, dc, 1024:1536],
                            start=(dc == 0), stop=(dc == DC - 1),
                        )
                    nc.scalar.activation(
                        v_sb[:, sc, :, 0:D],
                        ps[:].rearrange("p (h d) -> p h d", d=D),
                        ActCopy,
                    )
                # Q proj (fc 0-3)
                for fc in (0, 1, 2, 3):
                    if fc % 2 == 0:
                        ps = psp.tile([128, 1024], F32, tag="ps",
                                      name="ps")[:, 0:512]
                    else:
                        ps = qsdp.tile([128, 512], F32, tag="qsd", name="qsd")
                    for dc in range(DC):
                        nc.tensor.matmul(
                            ps[:],
                            wq_sb[:, dc, fc * 128:(fc + 1) * 128],
                            xts[sb][:, dc, :],
                            start=(dc == 0), stop=(dc == DC - 1),
                        )
                    rope(ps, fc, ssl)

            # ---------------- phase 2: attention + o_proj ----------------
            es_tiles = {}
            otp_tiles = {}

            # exp split: DVE Schraudolph singles at chunks {0,3,6,9,12,15}
            # (6/16 approximated), ACT table-exp on the remaining chunk
            # pairs (big ops amortize ACT's fixed overhead).  GPSIMD can't
            # read PSUM, so Pool gets no exp work.  Emission alternates
            # D,A,D,A,... so both engines stay fed and the two PSUM
            # rotations (qsd singles, ps pairs) are each deep enough.
            def emit_qk_exp(qb, h):
                qt, kt = qk_sb[h // 2], qk_sb[4 + h // 2]
                hp = (h % 2) * 64
                qsl = slice(qb * 512, (qb + 1) * 512)
                es = esp.tile([128, SC, 512], BF16, tag="es")
                es_tiles[(qb, h)] = es

                def qk_mm(qs_slice, c):
                    nc.tensor.matmul(
                        qs_slice,
                        kt[hp:hp + 64, c * 128:(c + 1) * 128],
                        qt[hp:hp + 64, qsl],
                        start=True, stop=True,
                    )

                for g in range(6):
                    cd = 3 * g           # 0, 3, 6, 9, 12, 15
                    qs = qsdp.tile([128, 512], F32, tag="qsd", name="qsd")
                    qk_mm(qs[:], cd)
                    nc.vector.tensor_scalar(
                        es[:, cd, :].bitcast(I16), qs[:],
                        A16 * 0.125, B16, MULT, ADD)
                    if g < 5:
                        ca = 3 * g + 1   # pairs (1,2),(4,5),(7,8),(10,11),(13,14)
                        qp = psp.tile([128, 1024], F32, tag="ps", name="qp")
                        qk_mm(qp[:, 0:512], ca)
                        qk_mm(qp[:, 512:1024], ca + 1)
                        dst2 = es[:, ca:ca + 2, :].rearrange("p a b -> p (a b)")
                        nc.scalar.activation(dst2, qp[:], Exp, scale=0.125)

            def emit_pv(qb, h, oproj_per_qsub=False):
                """pv[qpos, d+1] = sum_c es[c]^T @ v[c]; normalize with
                the ones-column denominator; DMA-transpose head pairs
                into o_proj lhsT layout."""
                es = es_tiles.pop((qb, h))
                hp2 = (h % 2) * 64
                if h % 2 == 0:
                    otp_tiles[qb] = otqp.tile([128, 4, 128], BF16,
                                              tag="otp", name=f"otp{qb}_{h}")
                otp = otp_tiles[qb]
                for qsub in range(4):
                    pvt = accp.tile([128, 512], F32, tag="acc", name="pvt")
                    qq = slice(qsub * 128, (qsub + 1) * 128)
                    for c in range(SC):
                        nc.tensor.matmul(
                            pvt[:, 0:D + 1],
                            es[:, c, qq],
                            v_sb[:, c, h, :],
                            start=(c == 0), stop=(c == SC - 1),
                        )
                    # normalize by the ones-column denominator: reciprocal
                    # to SBUF then per-partition scalar mult (DVE has no
                    # divide ISA op), PSUM -> bf16 SBUF
                    rc = rcp.tile([128, 1], F32, tag="rc")
                    nc.vector.reciprocal(rc[:], pvt[:, D:D + 1])
                    nc.vector.tensor_scalar(
                        otp[:, qsub, hp2:hp2 + D], pvt[:, 0:D],
                        rc[:], None, MULT)
                    if h % 2 == 1:
                        nc.sync.dma_start_transpose(
                            otT[h // 2][:, qb * 512 + qsub * 128:
                                        qb * 512 + (qsub + 1) * 128],
                            otp[:, qsub, :],
                        )
                        # trail by one qsub so the o_proj lhsT transpose
                        # latency hides behind the next PV group
                        if oproj_per_qsub and qsub > 0:
                            emit_oproj_s4(qb, qsub - 1)
                if oproj_per_qsub:
                    emit_oproj_s4(qb, 3)

            def emit_oproj_s4(qb, s4):
                sc = qb * 4 + s4
                og = ogp.tile([128, HD], F32, tag="og", name=f"og{sc}")
                for jb in range(2):
                    po = accp.tile([128, 512], F32, tag="acc", name="po")
                    for oc in range(4):
                        nc.tensor.matmul(
                            po[:],
                            otT[oc][:, sc * 128:(sc + 1) * 128],
                            ow_sb[:, oc, jb * 512:(jb + 1) * 512],
                            start=(oc == 0), stop=(oc == 3),
                        )
                    # ACT idles during o_proj windows (no exps there), so
                    # it takes the PSUM drain instead of DVE
                    nc.scalar.activation(
                        og[:, jb * 512:(jb + 1) * 512], po[:], ActCopy)
                    nc.sync.dma_start(
                        out_d[sc * 128:(sc + 1) * 128,
                              jb * 512:(jb + 1) * 512],
                        og[:, jb * 512:(jb + 1) * 512])

            def emit_oproj(qb):
                for s4 in range(4):
                    emit_oproj_s4(qb, s4)

            # one-block software pipeline: PV of block i runs while
            # exp of block i+1 is in flight.  PV (and its DVE
            # recip/normalize) is emitted BEFORE block i+1's exps so
            # the normalize isn't queued behind them on DVE.
            prev = None
            for qb in range(SB):
                for h in range(HG):
                    if prev is not None:
                        emit_pv(*prev)
                    emit_qk_exp(qb, h)
                    if prev is not None and prev[1] == HG - 1:
                        emit_oproj(prev[0])
                    prev = (qb, h)
            emit_pv(*prev, oproj_per_qsub=True)

    nc.compile()
    return nc


def kernel(hidden_states, qkv_w, o_w):
    global last_results
    from concourse.bass_utils import run_bass_kernel_spmd

    hidden_states = np.asarray(hidden_states, dtype=np.float32)
    qkv_w = np.asarray(qkv_w, dtype=np.float32)
    o_w = np.asarray(o_w, dtype=np.float32)

    cos128, sdn128, sup128 = _rope_tables()
    nc = _build()

    bf16 = ml_dtypes.bfloat16
    in_maps = []
    for core in range(N_CORES):
        b, g = core // G, core % G
        heads = range(g * HG, (g + 1) * HG)
        rows = np.concatenate(
              [np.arange(h * D, (h + 1) * D) for h in heads])
        wsel = np.concatenate(
              [qkv_w[off + rows] for off in (0, HD, 2 * HD)], axis=0)  # [1536,1024]
        in_maps.append({
              "xt": np.ascontiguousarray(hidden_states[b].T).astype(bf16),
              "wqkv": np.ascontiguousarray(wsel.T).astype(bf16),
              "owt": np.ascontiguousarray(o_w[:, rows].T).astype(bf16),
              "cos": cos128,
              "sdn": sdn128,
              "sup": sup128,
        })

    trace = bool(int(os.environ.get("KERNEL_TRACE", "0")))
    try:
        last_results = run_bass_kernel_spmd(
            nc, in_maps, core_ids=list(range(N_CORES)), trace=trace)
    except ModuleNotFoundError:
        # axon NTFF hook unavailable in this container; run without trace
        last_results = run_bass_kernel_spmd(
            nc, in_maps, core_ids=list(range(N_CORES)), trace=False)

    out = np.empty((B, S, HD), dtype=np.float32)
    for b in range(B):
        out[b] = last_results.results[b * G]["out"]
        for g in range(1, G):
              out[b] += last_results.results[b * G + g]["out"]
    return out
